# revision 15
# baseline (speedup 1.0000x reference)
"""ConvCaps dynamic-routing kernel for 8 TRN2 NeuronCores (v10).

Strategy (data-parallel over batch B=8, one batch element per core):
  - Host casts x / conv weights to bf16 (tolerance 2e-2 allows it):
    halves DMA traffic and runs the PE at full bf16 rate (fp32 is 1/4).
  - Grouped 3x3 conv (groups=D=32) as one matmul per group per pixel
    tile: stationary = im2col patches [72, npx], moving = weights
    [72, 512], PSUM -> u tile in SBUF [px, D, c, d] (bf16).
  - Routing iteration 1 is folded into the conv: s1 = sum_D c1[D,d]*u
    with c1 = softmax(prior) constant over (B, px), so a second matmul
    stream accumulates all 32 groups into one PSUM tile against
    host-prescaled weights (or the same weights * 1/32 when prior==0).
  - Remaining einsums (s = sum_D c*u, a = sum_c u*v) run as bf16
    tensor_tensor multiplies + tree-adds. bf16 TT gets the DVE 2x perf
    mode; TENSOR_REDUCE is capped at 1x, so trees beat reduces 2x.
    GpSimd takes a ~1/6 slice (it measures ~3.7ns/elem vs DVE 0.53).
  - All routing state is bf16 and double-buffered so adjacent pixel
    tiles overlap; sqrt is computed as exp(0.5*ln) so the Scalar engine
    stays on one activation table set (table switch costs 2.7us).
  - Output s3 (fp32) is PE-transposed to [(c,d), px] and DMA'd out.
"""

import numpy as np
from contextlib import ExitStack

import ml_dtypes

import concourse.bacc as bacc
import concourse.bass as bass
import concourse.tile as tile
import concourse.mybir as mybir
from concourse.bass_utils import run_bass_kernel_spmd
from concourse.masks import make_identity

F32 = mybir.dt.float32
BF16 = mybir.dt.bfloat16
NP_BF16 = ml_dtypes.bfloat16
AF = mybir.ActivationFunctionType

B = 8
C_IN, D_IN = 8, 32
C_OUT, D_OUT = 16, 32
KS = 3
H = W = 32
HO = WO = 30
NPX = HO * WO                 # 900 output pixels per batch element
KDIM = C_IN * KS * KS         # 72 = contraction dim of the conv matmul
CD = C_OUT * D_OUT            # 512 out-channels per group
P = 128
EPS = 1e-8
ROW_TILES = [(0, 4), (4, 4), (8, 4), (12, 4), (16, 4), (20, 4), (24, 4), (28, 2)]
MS = 28                       # D rows of the einsum muls on Vector (rest GpSimd)


def _mul_tree_D(nc, tmp, u_t, c_t, s_out, pxs, npx):
    """s_out[px,c,d] = sum_D c[px,D,d] * u[px,D,c,d] (bf16 mul + tree)."""
    nc.vector.tensor_mul(
        tmp[pxs], u_t[pxs],
        c_t[pxs].unsqueeze(2).broadcast_to((npx, D_IN, C_OUT, D_OUT)))
    # tree over D: 32 -> 16 -> 8 -> 4 -> 2 -> 1 (lvl1 on the DMA CCE)
    nc.vector.tensor_add(tmp[pxs, 0:16], tmp[pxs, 0:16], tmp[pxs, 16:32])
    nc.vector.tensor_add(tmp[pxs, 0:8], tmp[pxs, 0:8], tmp[pxs, 8:16])
    nc.vector.tensor_add(tmp[pxs, 0:4], tmp[pxs, 0:4], tmp[pxs, 4:8])
    nc.vector.tensor_add(tmp[pxs, 0:2], tmp[pxs, 0:2], tmp[pxs, 2:4])
    nc.vector.tensor_add(s_out[pxs], tmp[pxs, 0], tmp[pxs, 1])


def _mul_tree_c(nc, tmp, u_t, s_in, fb_t, b_t, ak_t, pxs, npx, accum,
                split_mul=False):
    """b[px,D,d] (+)= fb[px,d] * sum_c u[px,D,c,d] * s[px,c,d].

    The squash factor fb depends only on (px,d), so it is applied to the
    1K-element einsum result instead of making a 16K-element v tensor —
    and the squash scalar chain runs concurrently with the einsum.
    split_mul starts the multiply before the second half of u lands
    (first tile: hides half the conv latency)."""
    halves = [(0, D_IN // 2), (D_IN // 2, D_IN)] if split_mul \
        else [(0, D_IN)]
    for (lo, hi) in halves:
        nr_ = hi - lo
        nc.vector.tensor_mul(
            tmp[pxs, lo:hi], u_t[pxs, lo:hi],
            s_in[pxs].unsqueeze(1).broadcast_to((npx, nr_, C_OUT, D_OUT)))
        # tree over c: 16 -> 8 -> 4 -> 2 -> 1 (innermost d contiguous)
        nc.vector.tensor_add(tmp[pxs, lo:hi, 0:8], tmp[pxs, lo:hi, 0:8],
                             tmp[pxs, lo:hi, 8:16])
        nc.vector.tensor_add(tmp[pxs, lo:hi, 0:4], tmp[pxs, lo:hi, 0:4],
                             tmp[pxs, lo:hi, 4:8])
        nc.vector.tensor_add(tmp[pxs, lo:hi, 0:2], tmp[pxs, lo:hi, 0:2],
                             tmp[pxs, lo:hi, 2:4])
        nc.vector.tensor_add(ak_t[pxs, lo:hi], tmp[pxs, lo:hi, 0],
                             tmp[pxs, lo:hi, 1])
    if accum:
        nc.vector.tensor_mul(
            ak_t[pxs], ak_t[pxs],
            fb_t[pxs].unsqueeze(1).broadcast_to((npx, D_IN, D_OUT)))
        nc.vector.tensor_add(b_t[pxs], b_t[pxs], ak_t[pxs])
    else:
        nc.vector.tensor_mul(
            b_t[pxs], ak_t[pxs],
            fb_t[pxs].unsqueeze(1).broadcast_to((npx, D_IN, D_OUT)))


def _squash_factor(nc, s_in, sq_t, n2_t, r_t, f_t, fb_t, eps_ap, pxs, npx):
    """fb[px,d] (bf16) = n2 / ((1+n2) * sqrt(n2+eps)), n2 = sum_c s^2."""
    nc.scalar.square(sq_t[pxs], s_in[pxs])
    nc.vector.reduce_sum(n2_t[pxs], sq_t[pxs].transpose([0, 2, 1]),
                         axis=mybir.AxisListType.X)
    nc.scalar.activation(r_t[pxs], n2_t[pxs], AF.Sqrt, bias=eps_ap[pxs, 0:1])
    # f = (n2 + 1) * r ; fb = n2 / f  (bf16)
    nc.vector.scalar_tensor_tensor(f_t[pxs], n2_t[pxs], 1.0, r_t[pxs],
                                   op0=mybir.AluOpType.add,
                                   op1=mybir.AluOpType.mult)
    nc.vector.reciprocal(f_t[pxs], f_t[pxs])
    nc.vector.tensor_mul(fb_t[pxs], f_t[pxs], n2_t[pxs])


def _softmax(nc, b_t, e_t, ssum, rec_t, recb_t, pxs, npx):
    """c (bf16, in-place on e) = softmax over d of b[px,D,d]."""
    nc.scalar.activation(e_t[pxs], b_t[pxs], AF.Exp)
    nc.vector.reduce_sum(ssum[pxs], e_t[pxs], axis=mybir.AxisListType.X)
    nc.vector.reciprocal(rec_t[pxs], ssum[pxs])
    nc.scalar.copy(recb_t[pxs],
                   rec_t[pxs].unsqueeze(2).broadcast_to((npx, D_IN, D_OUT)))
    nc.vector.tensor_mul(e_t[pxs], e_t[pxs], recb_t[pxs])


def _im2col(nc, x9, xb, r0, nr):
    """9 shifted window loads, spread across both HWDGE queues."""
    idx = 0
    for kh in range(KS):
        for kw in range(KS):
            kk = kh * KS + kw
            for j in range(nr):
                # per-row copy keeps both DMA access patterns <= 3 dims
                eng = nc.sync if idx % 2 == 0 else nc.scalar
                idx += 1
                eng.dma_start(
                    x9[kk * C_IN:(kk + 1) * C_IN, :, j, :],
                    xb[:, :, r0 + kh + j, kw:kw + WO],
                )


def _body(ctx, tc, xb, wt, wt2, b0, out, zero_prior):
    nc = tc.nc
    consts = ctx.enter_context(tc.tile_pool(name="consts", bufs=1))
    wpool = ctx.enter_context(tc.tile_pool(name="wpool", bufs=1))
    x9pool = ctx.enter_context(tc.tile_pool(name="x9pool", bufs=2))
    upool = ctx.enter_context(tc.tile_pool(name="upool", bufs=2))
    tmppool = ctx.enter_context(
        tc.tile_pool(name="tmppool", bufs=2 if zero_prior else 1))
    rpool = ctx.enter_context(tc.tile_pool(name="rpool", bufs=2))
    opool = ctx.enter_context(tc.tile_pool(name="opool", bufs=2))
    psum_c = ctx.enter_context(tc.tile_pool(name="psum_c", bufs=4, space="PSUM"))
    psum_s = ctx.enter_context(tc.tile_pool(name="psum_s", bufs=2, space="PSUM"))
    psum_t = ctx.enter_context(tc.tile_pool(name="psum_t", bufs=2, space="PSUM"))

    # tile-0 im2col goes out first so the 2.25MB weight DMA does not
    # head-of-line block it on the sync queue
    x9_first = x9pool.tile([KDIM, D_IN, 4, WO], BF16, tag="x9")
    _im2col(nc, x9_first, xb, 0, ROW_TILES[0][1])
    w_sb = wpool.tile([KDIM, D_IN * CD], BF16)
    nc.sync.dma_start(w_sb[0:KDIM // 2], wt[0:KDIM // 2])
    nc.scalar.dma_start(w_sb[KDIM // 2:], wt[KDIM // 2:])
    if zero_prior:
        w2_sb = w_sb
    else:
        w2_sb = wpool.tile([KDIM, D_IN * CD], BF16)
        nc.sync.dma_start(w2_sb[:], wt2)
    ident = consts.tile([P, P], F32)
    make_identity(nc, ident)
    eps_ap = consts.tile([P, 1], F32)
    nc.gpsimd.memset(eps_ap[:], EPS)
    # warm the PE (HAM pstate) right before tile-0's conv: a ~4us chain of
    # tiny matmuls gated on the weight load, so the real conv runs at the
    # 2.4GHz warm clock instead of the 1.2GHz cold one
    for _ in range(28):
        dp = psum_t.tile([P, 120], F32, tag="pt")
        nc.tensor.matmul(dp[0:1, 0:1], w_sb[0:1, 0:1], w_sb[0:1, 0:1],
                         start=True, stop=True)
    if not zero_prior:
        b0_sb = consts.tile([P, D_IN, D_OUT], BF16)
        nc.sync.dma_start(b0_sb[:], b0)

    for (r0, nr) in ROW_TILES:
        npx = nr * WO
        pxs = slice(0, npx)

        # ---- im2col: 9 shifted window loads; partition k = (kh*3+kw)*8 + C
        if r0 == 0:
            x9 = x9_first
        else:
            x9 = x9pool.tile([KDIM, D_IN, 4, WO], BF16, tag="x9")
            _im2col(nc, x9, xb, r0, nr)

        # ---- grouped conv + folded first routing iteration
        u_t = upool.tile([P, D_IN, C_OUT, D_OUT], BF16, tag="u")
        s1p = psum_s.tile([P, CD], F32, tag="s1p")
        for g in range(D_IN):
            nc.tensor.matmul(
                s1p[pxs, :],
                x9[:, g, 0:nr, :],
                w2_sb[:, g * CD:(g + 1) * CD],
                start=(g == 0), stop=(g == D_IN - 1),
            )
        for g in range(D_IN):
            pu = psum_c.tile([P, CD], F32, tag="pu")
            nc.tensor.matmul(
                pu[pxs, :],
                x9[:, g, 0:nr, :],
                w_sb[:, g * CD:(g + 1) * CD],
                start=True, stop=True,
            )
            nc.scalar.copy(u_t[pxs, g], pu[pxs, :])

        # ---- routing state tiles (double-buffered for cross-tile overlap)
        b_t = rpool.tile([P, D_IN, D_OUT], BF16, tag="b")
        e_t = rpool.tile([P, D_IN, D_OUT], BF16, tag="e")
        ak_t = rpool.tile([P, D_IN, D_OUT], BF16, tag="ak")
        s_b = rpool.tile([P, C_OUT, D_OUT], BF16, tag="sb")
        s_f = rpool.tile([P, C_OUT, D_OUT], F32, tag="sf")
        sq_t = rpool.tile([P, C_OUT, D_OUT], BF16, tag="sq")
        n2_t = rpool.tile([P, D_OUT], F32, tag="n2")
        r_t = rpool.tile([P, D_OUT], F32, tag="r")
        f_t = rpool.tile([P, D_OUT], F32, tag="f")
        fb_t = rpool.tile([P, D_OUT], BF16, tag="fb")
        ssum = rpool.tile([P, D_IN], F32, tag="ssum")
        rec_t = rpool.tile([P, D_IN], F32, tag="rec")
        recb_t = rpool.tile([P, D_IN, D_OUT], BF16, tag="recb")

        # s1 from the PE-accumulated stream (c1 folded into w2 / the scale)
        nc.scalar.mul(s_b[pxs], s1p[pxs], 1.0 / D_IN if zero_prior else 1.0)

        tmp = tmppool.tile([P, D_IN, C_OUT, D_OUT], BF16, tag="tmp")

        # iteration 1 tail: fb1 = squash_factor(s1); b1 = b0 + fb1*sum_c u*s1
        _squash_factor(nc, s_b, sq_t, n2_t, r_t, f_t, fb_t, eps_ap, pxs, npx)
        if zero_prior:
            _mul_tree_c(nc, tmp, u_t, s_b, fb_t, b_t, ak_t, pxs, npx,
                        accum=False, split_mul=(r0 == 0))
        else:
            nc.scalar.copy(b_t[pxs], b0_sb[pxs])
            _mul_tree_c(nc, tmp, u_t, s_b, fb_t, b_t, ak_t, pxs, npx,
                        accum=True)

        # iteration 2: c2 = softmax(b1); s2; b2 = b1 + fb2*sum_c u*s2
        _softmax(nc, b_t, e_t, ssum, rec_t, recb_t, pxs, npx)
        _mul_tree_D(nc, tmp, u_t, e_t, s_b, pxs, npx)
        _squash_factor(nc, s_b, sq_t, n2_t, r_t, f_t, fb_t, eps_ap, pxs, npx)
        _mul_tree_c(nc, tmp, u_t, s_b, fb_t, b_t, ak_t, pxs, npx, accum=True)

        # iteration 3: c3 = softmax(b2); s3 (the output, fp32)
        _softmax(nc, b_t, e_t, ssum, rec_t, recb_t, pxs, npx)
        _mul_tree_D(nc, tmp, u_t, e_t, s_f, pxs, npx)

        # ---- write s3 out as [(c,d), px]: PE transpose in 128-row blocks
        s_flat = s_f[:].rearrange("p a b -> p (a b)")
        for blk in range(CD // P):
            pt = psum_t.tile([P, 120], F32, tag="pt")
            nc.tensor.transpose(
                pt[:, pxs], s_flat[pxs, blk * P:(blk + 1) * P],
                ident[pxs, pxs])
            ob = opool.tile([P, 120], F32, tag="ob")
            nc.scalar.copy(ob[:, pxs], pt[:, pxs])
            nc.sync.dma_start(
                out[blk * P:(blk + 1) * P, r0 * WO:r0 * WO + npx],
                ob[:, pxs])


_CACHE = {}


def _build(zero_prior: bool):
    key = ("v10", zero_prior)
    if key in _CACHE:
        return _CACHE[key]
    nc = bacc.Bacc("TRN2", target_bir_lowering=False, debug=False,
                   enable_asserts=True, num_devices=B)
    xb = nc.dram_tensor("xb", [C_IN, D_IN, H, W], BF16,
                        kind="ExternalInput").ap()
    wt = nc.dram_tensor("wt", [KDIM, D_IN * CD], BF16,
                        kind="ExternalInput").ap()
    wt2 = b0 = None
    if not zero_prior:
        wt2 = nc.dram_tensor("wt2", [KDIM, D_IN * CD], BF16,
                             kind="ExternalInput").ap()
        b0 = nc.dram_tensor("b0", [P, D_IN, D_OUT], BF16,
                            kind="ExternalInput").ap()
    out = nc.dram_tensor("out", [CD, NPX], F32, kind="ExternalOutput").ap()
    with tile.TileContext(nc) as tc:
        with ExitStack() as ctx:
            _body(ctx, tc, xb, wt, wt2, b0, out, zero_prior)
    nc.compile()
    _CACHE[key] = nc
    return nc


def _prep_inputs(x, conv_w, prior):
    zero_prior = not np.any(prior)
    # weights: rows (D,c,d) x (C,kh,kw) -> [k=(kh,kw,C), (D,c,d)]
    w6 = conv_w.reshape(D_IN, C_OUT, D_OUT, C_IN, KS, KS)
    wt = np.ascontiguousarray(w6.transpose(4, 5, 3, 0, 1, 2)) \
        .reshape(KDIM, D_IN * CD).astype(NP_BF16)
    base = {"wt": wt}
    if not zero_prior:
        pr = prior.reshape(D_IN, D_OUT).astype(np.float64)
        e = np.exp(pr - pr.max(axis=1, keepdims=True))
        c1 = (e / e.sum(axis=1, keepdims=True)).astype(np.float32)
        w6s = w6 * c1[:, None, :, None, None, None]
        wt2 = np.ascontiguousarray(w6s.transpose(4, 5, 3, 0, 1, 2)) \
            .reshape(KDIM, D_IN * CD).astype(NP_BF16)
        b0 = np.ascontiguousarray(np.broadcast_to(
            prior.reshape(D_IN, D_OUT), (P, D_IN, D_OUT))).astype(NP_BF16)
        base["wt2"] = wt2
        base["b0"] = b0
    in_maps = [
        {"xb": np.ascontiguousarray(x[b]).astype(NP_BF16), **base}
        for b in range(B)
    ]
    return in_maps


def kernel(x, conv_w, prior):
    x = np.asarray(x, dtype=np.float32)
    conv_w = np.asarray(conv_w, dtype=np.float32)
    prior = np.asarray(prior, dtype=np.float32)
    zero_prior = not np.any(prior)
    nc = _build(zero_prior)
    in_maps = _prep_inputs(x, conv_w, prior)
    res = run_bass_kernel_spmd(nc, in_maps, list(range(B)))
    outs = [res.results[b]["out"].reshape(C_OUT, D_OUT, HO, WO)
            for b in range(B)]
    return np.stack(outs, axis=0).astype(np.float32)


# revision 16
# speedup vs baseline: 1.0287x; 1.0287x over previous
"""ConvCaps dynamic-routing kernel for 8 TRN2 NeuronCores (v9).

Strategy (data-parallel over batch B=8, one batch element per core):
  - Host casts x / conv weights to bf16 (tolerance 2e-2 allows it):
    halves DMA traffic and runs the PE at full bf16 rate (fp32 is 1/4).
  - Grouped 3x3 conv (groups=D=32) as one matmul per group per pixel
    tile: stationary = im2col patches [72, npx], moving = weights
    [72, 512], PSUM -> u tile in SBUF [px, D, c, d] (bf16).
  - Routing iteration 1 is folded into the conv: s1 = sum_D c1[D,d]*u
    with c1 = softmax(prior) constant over (B, px), so a second matmul
    stream accumulates all 32 groups into one PSUM tile against
    host-prescaled weights (or the same weights * 1/32 when prior==0).
  - Remaining einsums (s = sum_D c*u, a = sum_c u*v) run as bf16
    tensor_tensor multiplies + tree-adds. bf16 TT gets the DVE 2x perf
    mode; TENSOR_REDUCE is capped at 1x, so trees beat reduces 2x.
    GpSimd takes a ~1/6 slice (it measures ~3.7ns/elem vs DVE 0.53).
  - All routing state is bf16 and double-buffered so adjacent pixel
    tiles overlap; sqrt is computed as exp(0.5*ln) so the Scalar engine
    stays on one activation table set (table switch costs 2.7us).
  - Output s3 (fp32) is PE-transposed to [(c,d), px] and DMA'd out.
"""

import numpy as np
from contextlib import ExitStack

import ml_dtypes

import concourse.bacc as bacc
import concourse.bass as bass
import concourse.tile as tile
import concourse.mybir as mybir
from concourse.bass_utils import run_bass_kernel_spmd
from concourse.masks import make_identity

F32 = mybir.dt.float32
BF16 = mybir.dt.bfloat16
NP_BF16 = ml_dtypes.bfloat16
AF = mybir.ActivationFunctionType

B = 8
C_IN, D_IN = 8, 32
C_OUT, D_OUT = 16, 32
KS = 3
H = W = 32
HO = WO = 30
NPX = HO * WO                 # 900 output pixels per batch element
KDIM = C_IN * KS * KS         # 72 = contraction dim of the conv matmul
CD = C_OUT * D_OUT            # 512 out-channels per group
P = 128
EPS = 1e-8
ROW_TILES = [(0, 4), (4, 4), (8, 4), (12, 4), (16, 4), (20, 4), (24, 4), (28, 2)]
MS = 28                       # D rows of the einsum muls on Vector (rest GpSimd)


def _mul_tree_D(nc, tmp, u_t, c_t, s_out, pxs, npx, chunk_last=False):
    """s_out[px,c,d] = sum_D c[px,D,d] * u[px,D,c,d] (bf16 mul + tree)."""
    nc.vector.tensor_mul(
        tmp[pxs], u_t[pxs],
        c_t[pxs].unsqueeze(2).broadcast_to((npx, D_IN, C_OUT, D_OUT)))
    # tree over D: 32 -> 16 -> 8 -> 4 -> 2 -> 1 (lvl1 on the DMA CCE)
    nc.vector.tensor_add(tmp[pxs, 0:16], tmp[pxs, 0:16], tmp[pxs, 16:32])
    nc.vector.tensor_add(tmp[pxs, 0:8], tmp[pxs, 0:8], tmp[pxs, 8:16])
    nc.vector.tensor_add(tmp[pxs, 0:4], tmp[pxs, 0:4], tmp[pxs, 4:8])
    nc.vector.tensor_add(tmp[pxs, 0:2], tmp[pxs, 0:2], tmp[pxs, 2:4])
    if chunk_last:
        for k in range(4):
            nc.vector.tensor_add(s_out[pxs, 4 * k:4 * k + 4],
                                 tmp[pxs, 0, 4 * k:4 * k + 4],
                                 tmp[pxs, 1, 4 * k:4 * k + 4])
    else:
        nc.vector.tensor_add(s_out[pxs], tmp[pxs, 0], tmp[pxs, 1])


def _mul_tree_c(nc, tmp, u_t, s_in, fb_t, b_t, ak_t, pxs, npx, accum,
                split_mul=False):
    """b[px,D,d] (+)= fb[px,d] * sum_c u[px,D,c,d] * s[px,c,d].

    The squash factor fb depends only on (px,d), so it is applied to the
    1K-element einsum result instead of making a 16K-element v tensor —
    and the squash scalar chain runs concurrently with the einsum.
    split_mul starts the multiply before the second half of u lands
    (first tile: hides half the conv latency)."""
    halves = [(0, D_IN // 2), (D_IN // 2, D_IN)] if split_mul \
        else [(0, D_IN)]
    for (lo, hi) in halves:
        nr_ = hi - lo
        nc.vector.tensor_mul(
            tmp[pxs, lo:hi], u_t[pxs, lo:hi],
            s_in[pxs].unsqueeze(1).broadcast_to((npx, nr_, C_OUT, D_OUT)))
        # tree over c: 16 -> 8 -> 4 -> 2 -> 1 (innermost d contiguous)
        nc.vector.tensor_add(tmp[pxs, lo:hi, 0:8], tmp[pxs, lo:hi, 0:8],
                             tmp[pxs, lo:hi, 8:16])
        nc.vector.tensor_add(tmp[pxs, lo:hi, 0:4], tmp[pxs, lo:hi, 0:4],
                             tmp[pxs, lo:hi, 4:8])
        nc.vector.tensor_add(tmp[pxs, lo:hi, 0:2], tmp[pxs, lo:hi, 0:2],
                             tmp[pxs, lo:hi, 2:4])
        nc.vector.tensor_add(ak_t[pxs, lo:hi], tmp[pxs, lo:hi, 0],
                             tmp[pxs, lo:hi, 1])
    if accum:
        nc.vector.tensor_mul(
            ak_t[pxs], ak_t[pxs],
            fb_t[pxs].unsqueeze(1).broadcast_to((npx, D_IN, D_OUT)))
        nc.vector.tensor_add(b_t[pxs], b_t[pxs], ak_t[pxs])
    else:
        nc.vector.tensor_mul(
            b_t[pxs], ak_t[pxs],
            fb_t[pxs].unsqueeze(1).broadcast_to((npx, D_IN, D_OUT)))


def _squash_factor(nc, s_in, sq_t, n2_t, r_t, f_t, fb_t, eps_ap, pxs, npx):
    """fb[px,d] (bf16) = n2 / ((1+n2) * sqrt(n2+eps)), n2 = sum_c s^2."""
    nc.scalar.square(sq_t[pxs], s_in[pxs])
    nc.vector.reduce_sum(n2_t[pxs], sq_t[pxs].transpose([0, 2, 1]),
                         axis=mybir.AxisListType.X)
    nc.scalar.activation(r_t[pxs], n2_t[pxs], AF.Sqrt, bias=eps_ap[pxs, 0:1])
    # f = (n2 + 1) * r ; fb = n2 / f  (bf16)
    nc.vector.scalar_tensor_tensor(f_t[pxs], n2_t[pxs], 1.0, r_t[pxs],
                                   op0=mybir.AluOpType.add,
                                   op1=mybir.AluOpType.mult)
    nc.vector.reciprocal(f_t[pxs], f_t[pxs])
    nc.vector.tensor_mul(fb_t[pxs], f_t[pxs], n2_t[pxs])


def _softmax(nc, b_t, e_t, ssum, rec_t, recb_t, pxs, npx):
    """c (bf16, in-place on e) = softmax over d of b[px,D,d]."""
    nc.scalar.activation(e_t[pxs], b_t[pxs], AF.Exp)
    nc.vector.reduce_sum(ssum[pxs], e_t[pxs], axis=mybir.AxisListType.X)
    nc.vector.reciprocal(rec_t[pxs], ssum[pxs])
    nc.scalar.copy(recb_t[pxs],
                   rec_t[pxs].unsqueeze(2).broadcast_to((npx, D_IN, D_OUT)))
    nc.vector.tensor_mul(e_t[pxs], e_t[pxs], recb_t[pxs])


def _body(ctx, tc, xb, wt, wt2, b0, out, zero_prior):
    nc = tc.nc
    consts = ctx.enter_context(tc.tile_pool(name="consts", bufs=1))
    wpool = ctx.enter_context(tc.tile_pool(name="wpool", bufs=1))
    x9pool = ctx.enter_context(tc.tile_pool(name="x9pool", bufs=2))
    upool = ctx.enter_context(tc.tile_pool(name="upool", bufs=2))
    tmppool = ctx.enter_context(
        tc.tile_pool(name="tmppool", bufs=2 if zero_prior else 1))
    rpool = ctx.enter_context(tc.tile_pool(name="rpool", bufs=2))
    opool = ctx.enter_context(tc.tile_pool(name="opool", bufs=2))
    psum_c = ctx.enter_context(tc.tile_pool(name="psum_c", bufs=4, space="PSUM"))
    psum_s = ctx.enter_context(tc.tile_pool(name="psum_s", bufs=2, space="PSUM"))
    psum_t = ctx.enter_context(tc.tile_pool(name="psum_t", bufs=2, space="PSUM"))

    w_sb = wpool.tile([KDIM, D_IN * CD], BF16)
    nc.sync.dma_start(w_sb[:], wt)
    if zero_prior:
        w2_sb = w_sb
    else:
        w2_sb = wpool.tile([KDIM, D_IN * CD], BF16)
        nc.sync.dma_start(w2_sb[:], wt2)
    ident = consts.tile([P, P], F32)
    make_identity(nc, ident)
    eps_ap = consts.tile([P, 1], F32)
    nc.gpsimd.memset(eps_ap[:], EPS)
    if not zero_prior:
        b0_sb = consts.tile([P, D_IN, D_OUT], BF16)
        nc.sync.dma_start(b0_sb[:], b0)

    for (r0, nr) in ROW_TILES:
        npx = nr * WO
        pxs = slice(0, npx)

        # ---- im2col: 9 shifted window loads; partition k = (kh*3+kw)*8 + C
        # startup is bound by tile-0's x9 DMAs (~1.7us effective each), so
        # the first tile spreads them over three queues (incl gpsimd SWDGE)
        x9 = x9pool.tile([KDIM, D_IN, 4, WO], BF16, tag="x9")
        for kh in range(KS):
            for kw in range(KS):
                kk = kh * KS + kw
                if r0 == 0:
                    eng = (nc.sync, nc.scalar, nc.gpsimd)[kh]
                else:
                    eng = nc.scalar if kh == 1 else nc.sync
                for j in range(nr):
                    # per-row copy keeps both DMA access patterns <= 3 dims
                    eng.dma_start(
                        x9[kk * C_IN:(kk + 1) * C_IN, :, j, :],
                        xb[:, :, r0 + kh + j, kw:kw + WO],
                    )

        # ---- grouped conv + folded first routing iteration
        u_t = upool.tile([P, D_IN, C_OUT, D_OUT], BF16, tag="u")
        s1p = psum_s.tile([P, CD], F32, tag="s1p")
        for g in range(D_IN):
            nc.tensor.matmul(
                s1p[pxs, :],
                x9[:, g, 0:nr, :],
                w2_sb[:, g * CD:(g + 1) * CD],
                start=(g == 0), stop=(g == D_IN - 1),
            )
        for g in range(D_IN):
            pu = psum_c.tile([P, CD], F32, tag="pu")
            nc.tensor.matmul(
                pu[pxs, :],
                x9[:, g, 0:nr, :],
                w_sb[:, g * CD:(g + 1) * CD],
                start=True, stop=True,
            )
            nc.scalar.copy(u_t[pxs, g], pu[pxs, :])

        # ---- routing state tiles (double-buffered for cross-tile overlap)
        b_t = rpool.tile([P, D_IN, D_OUT], BF16, tag="b")
        e_t = rpool.tile([P, D_IN, D_OUT], BF16, tag="e")
        ak_t = rpool.tile([P, D_IN, D_OUT], BF16, tag="ak")
        s_b = rpool.tile([P, C_OUT, D_OUT], BF16, tag="sb")
        s_f = rpool.tile([P, C_OUT, D_OUT], F32, tag="sf")
        sq_t = rpool.tile([P, C_OUT, D_OUT], BF16, tag="sq")
        n2_t = rpool.tile([P, D_OUT], F32, tag="n2")
        r_t = rpool.tile([P, D_OUT], F32, tag="r")
        f_t = rpool.tile([P, D_OUT], F32, tag="f")
        fb_t = rpool.tile([P, D_OUT], BF16, tag="fb")
        ssum = rpool.tile([P, D_IN], F32, tag="ssum")
        rec_t = rpool.tile([P, D_IN], F32, tag="rec")
        recb_t = rpool.tile([P, D_IN, D_OUT], BF16, tag="recb")

        # s1 from the PE-accumulated stream (c1 folded into w2 / the scale)
        nc.scalar.mul(s_b[pxs], s1p[pxs], 1.0 / D_IN if zero_prior else 1.0)

        tmp = tmppool.tile([P, D_IN, C_OUT, D_OUT], BF16, tag="tmp")

        # iteration 1 tail: fb1 = squash_factor(s1); b1 = b0 + fb1*sum_c u*s1
        _squash_factor(nc, s_b, sq_t, n2_t, r_t, f_t, fb_t, eps_ap, pxs, npx)
        if zero_prior:
            _mul_tree_c(nc, tmp, u_t, s_b, fb_t, b_t, ak_t, pxs, npx,
                        accum=False, split_mul=(r0 == 0))
        else:
            nc.scalar.copy(b_t[pxs], b0_sb[pxs])
            _mul_tree_c(nc, tmp, u_t, s_b, fb_t, b_t, ak_t, pxs, npx,
                        accum=True)

        # iteration 2: c2 = softmax(b1); s2; b2 = b1 + fb2*sum_c u*s2
        _softmax(nc, b_t, e_t, ssum, rec_t, recb_t, pxs, npx)
        _mul_tree_D(nc, tmp, u_t, e_t, s_b, pxs, npx)
        _squash_factor(nc, s_b, sq_t, n2_t, r_t, f_t, fb_t, eps_ap, pxs, npx)
        _mul_tree_c(nc, tmp, u_t, s_b, fb_t, b_t, ak_t, pxs, npx, accum=True)

        # iteration 3: c3 = softmax(b2); s3 (the output, fp32)
        _softmax(nc, b_t, e_t, ssum, rec_t, recb_t, pxs, npx)
        _mul_tree_D(nc, tmp, u_t, e_t, s_f, pxs, npx,
                    chunk_last=(r0 == ROW_TILES[-1][0]))

        # ---- write s3 out as [(c,d), px]: PE transpose in 128-row blocks
        s_flat = s_f[:].rearrange("p a b -> p (a b)")
        for blk in range(CD // P):
            pt = psum_t.tile([P, 120], F32, tag="pt")
            nc.tensor.transpose(
                pt[:, pxs], s_flat[pxs, blk * P:(blk + 1) * P],
                ident[pxs, pxs])
            ob = opool.tile([P, 120], F32, tag="ob")
            nc.scalar.copy(ob[:, pxs], pt[:, pxs])
            nc.sync.dma_start(
                out[blk * P:(blk + 1) * P, r0 * WO:r0 * WO + npx],
                ob[:, pxs])


_CACHE = {}


def _build(zero_prior: bool):
    key = ("v11", zero_prior)
    if key in _CACHE:
        return _CACHE[key]
    nc = bacc.Bacc("TRN2", target_bir_lowering=False, debug=False,
                   enable_asserts=True, num_devices=B)
    xb = nc.dram_tensor("xb", [C_IN, D_IN, H, W], BF16,
                        kind="ExternalInput").ap()
    wt = nc.dram_tensor("wt", [KDIM, D_IN * CD], BF16,
                        kind="ExternalInput").ap()
    wt2 = b0 = None
    if not zero_prior:
        wt2 = nc.dram_tensor("wt2", [KDIM, D_IN * CD], BF16,
                             kind="ExternalInput").ap()
        b0 = nc.dram_tensor("b0", [P, D_IN, D_OUT], BF16,
                            kind="ExternalInput").ap()
    out = nc.dram_tensor("out", [CD, NPX], F32, kind="ExternalOutput").ap()
    with tile.TileContext(nc) as tc:
        with ExitStack() as ctx:
            _body(ctx, tc, xb, wt, wt2, b0, out, zero_prior)
    nc.compile()
    _CACHE[key] = nc
    return nc


def _prep_inputs(x, conv_w, prior):
    zero_prior = not np.any(prior)
    # weights: rows (D,c,d) x (C,kh,kw) -> [k=(kh,kw,C), (D,c,d)]
    w6 = conv_w.reshape(D_IN, C_OUT, D_OUT, C_IN, KS, KS)
    wt = np.ascontiguousarray(w6.transpose(4, 5, 3, 0, 1, 2)) \
        .reshape(KDIM, D_IN * CD).astype(NP_BF16)
    base = {"wt": wt}
    if not zero_prior:
        pr = prior.reshape(D_IN, D_OUT).astype(np.float64)
        e = np.exp(pr - pr.max(axis=1, keepdims=True))
        c1 = (e / e.sum(axis=1, keepdims=True)).astype(np.float32)
        w6s = w6 * c1[:, None, :, None, None, None]
        wt2 = np.ascontiguousarray(w6s.transpose(4, 5, 3, 0, 1, 2)) \
            .reshape(KDIM, D_IN * CD).astype(NP_BF16)
        b0 = np.ascontiguousarray(np.broadcast_to(
            prior.reshape(D_IN, D_OUT), (P, D_IN, D_OUT))).astype(NP_BF16)
        base["wt2"] = wt2
        base["b0"] = b0
    in_maps = [
        {"xb": np.ascontiguousarray(x[b]).astype(NP_BF16), **base}
        for b in range(B)
    ]
    return in_maps


def kernel(x, conv_w, prior):
    x = np.asarray(x, dtype=np.float32)
    conv_w = np.asarray(conv_w, dtype=np.float32)
    prior = np.asarray(prior, dtype=np.float32)
    zero_prior = not np.any(prior)
    nc = _build(zero_prior)
    in_maps = _prep_inputs(x, conv_w, prior)
    res = run_bass_kernel_spmd(nc, in_maps, list(range(B)))
    outs = [res.results[b]["out"].reshape(C_OUT, D_OUT, HO, WO)
            for b in range(B)]
    return np.stack(outs, axis=0).astype(np.float32)
